# revision 1
# baseline (speedup 1.0000x reference)
"""ComplexDenseSO2 Trainium2 kernel.

Computes out = (X @ conj(B)^T * w) @ B for complex X [64, 32400],
B [2048, 32400], w [2048], given as separate re/im fp32 planes.

Strategy (tensor-parallel over D across 8 cores):
  - Fold w into the first-matmul operand on the host:
    M = diag(w) @ conj(B), so mm1 output IS Y = X @ M^T.
  - Pad D 32400 -> 32768; core c owns d-slice [c*4096, (c+1)*4096).
  - mm1 (per core): stationary [Xr|Xi]^T d-tiles [128,128], moving
    M^T d-tiles [128, 512k]; PSUM accumulates over d -> partial Y
    in [j=128, k] layout (j: 0:64 real-X rows, 64:128 imag-X rows).
  - PE-transpose to k-major, combine re/im parts (free-dim slices),
    DMA to a DRAM bounce -> AllReduce(add) over the 8 cores (1 MB).
  - Post-AR: build fp16 stationaries YtA = [Yr|Yi], YtB = [-Yi|Yr].
  - mm2: out[128, d] PSUM accumulates YtA^T@Br' + YtB^T@Bi' over k,
    which yields rows 0:64 = Or, rows 64:128 = Oi directly.
  - fp16 operands use power-of-2 prescales (M*1024, B*256) to stay
    clear of fp16 subnormals; the epilogue descales by 2^-18.
"""

import sys

if "/opt/trn_rl_repo" not in sys.path:
    sys.path.insert(0, "/opt/trn_rl_repo")

import numpy as np

B_, K, D = 64, 2048, 32400
NCORES = 8
DP = 32768
DL = DP // NCORES  # 4096

COMPUTE_DT = "float16"  # or "bfloat16"
SCALE_M = 1024.0
SCALE_B = 256.0

_nc_cache = {}


def build_nc(n_cores=NCORES, k=K, dl=DL):
    import concourse.mybir as mybir
    from concourse import bacc
    import concourse.tile as tile
    from concourse.masks import make_identity

    fp = getattr(mybir.dt, COMPUTE_DT)
    f32 = mybir.dt.float32

    ndt = dl // 128  # mm1 d-tiles
    nkc = k // 512   # mm1 k-chunks
    nkb = k // 128   # k-blocks
    ndc = dl // 512  # mm2 d-chunks

    nc = bacc.Bacc(
        trn_type="TRN2",
        target_bir_lowering=False,
        debug=False,
        num_devices=n_cores,
    )
    xt = nc.dram_tensor("xt", [dl, 128], fp, kind="ExternalInput")
    mtr = nc.dram_tensor("mtr", [dl, k], fp, kind="ExternalInput")
    mti = nc.dram_tensor("mti", [dl, k], fp, kind="ExternalInput")
    bnr = nc.dram_tensor("bnr", [k, dl], fp, kind="ExternalInput")
    bni = nc.dram_tensor("bni", [k, dl], fp, kind="ExternalInput")
    out = nc.dram_tensor("out", [128, dl], f32, kind="ExternalOutput")

    with tile.TileContext(nc) as tc:
        with (
            tc.tile_pool(name="sb", bufs=2) as sb,
            tc.tile_pool(name="sbx", bufs=1) as sbx,
            tc.tile_pool(name="ps", bufs=1, space="PSUM") as ps,
            tc.tile_pool(name="dram", bufs=1, space="DRAM") as dram,
        ):
            ident = sbx.tile([128, 128], f32, tag="ident")
            make_identity(nc, ident)

            xts_all = sbx.tile([128, dl], fp, tag="xts_all", name="xts_all")
            nc.sync.dma_start(
                out=xts_all.rearrange("p (t j) -> p t j", j=128),
                in_=xt.ap().rearrange("(t p) j -> p t j", p=128),
            )
            xts = [xts_all[:, dt * 128 : (dt + 1) * 128] for dt in range(ndt)]

            arin = dram.tile([k, 128], fp, tag="arin", name="arin")
            arout = dram.tile(
                [k, 128], fp, tag="arout", name="arout", addr_space="Shared"
            )

            # ---------------- mm1 ----------------
            # Two passes over d, each covering a 1024-wide k-chunk pair.
            # Per d-tile one 256KB DMA per component; 4 matmuls share the
            # stationary x-tile. 4 PSUM accumulator banks per pass.
            kw = min(1024, k)
            nq = kw // 512
            for half in range(k // kw):
                ks = slice(half * kw, (half + 1) * kw)
                accs = []
                for q in range(nq):
                    a_r = ps.tile([128, 512], f32, tag=f"a{q}r", name=f"a{q}r")
                    a_i = ps.tile([128, 512], f32, tag=f"a{q}i", name=f"a{q}i")
                    accs.append((a_r, a_i))
                for dt in range(ndt):
                    rs = slice(dt * 128, (dt + 1) * 128)
                    mr_t = sb.tile([128, kw], fp, tag="mr", name="mr", bufs=10)
                    nc.sync.dma_start(out=mr_t, in_=mtr[rs, ks])
                    mi_t = sb.tile([128, kw], fp, tag="mi", name="mi", bufs=10)
                    nc.sync.dma_start(out=mi_t, in_=mti[rs, ks])
                    st, sp = dt == 0, dt == ndt - 1
                    for q in range(nq):
                        qs = slice(q * 512, (q + 1) * 512)
                        nc.tensor.matmul(accs[q][0], lhsT=xts[dt], rhs=mr_t[:, qs], start=st, stop=sp)
                        nc.tensor.matmul(accs[q][1], lhsT=xts[dt], rhs=mi_t[:, qs], start=st, stop=sp)
                for q in range(nq):
                    pscr = sb.tile([128, 512], f32, tag="pscr", name="pscr", bufs=2)
                    nc.vector.tensor_copy(pscr, accs[q][0])
                    psci = sb.tile([128, 512], f32, tag="psci", name="psci", bufs=2)
                    nc.scalar.copy(psci, accs[q][1])
                    for j in range(4):
                        kb = half * (4 * nq) + q * 4 + j
                        js = slice(j * 128, (j + 1) * 128)
                        tp_r = ps.tile([128, 128], f32, tag="tp_r", name="tp_r", bufs=1)
                        nc.tensor.transpose(tp_r, pscr[:, js], ident)
                        tp_i = ps.tile([128, 128], f32, tag="tp_i", name="tp_i", bufs=1)
                        nc.tensor.transpose(tp_i, psci[:, js], ident)
                        # DVE tensor_tensor may read only one PSUM input:
                        # evacuate tp_r to SBUF, combine against tp_i in PSUM.
                        cc_r = sb.tile([128, 128], f32, tag="cc_r", name="cc_r", bufs=3)
                        nc.vector.tensor_copy(cc_r, tp_r)
                        c_kb = sb.tile([128, 128], fp, tag="c_kb", name="c_kb", bufs=4)
                        # Yr = re(X@Mr^T) - im(X@Mi^T); Yi = im(X@Mr^T) + re(X@Mi^T)
                        nc.vector.tensor_sub(c_kb[:, 0:64], cc_r[:, 0:64], tp_i[:, 64:128])
                        nc.vector.tensor_add(c_kb[:, 64:128], cc_r[:, 64:128], tp_i[:, 0:64])
                        nc.sync.dma_start(out=arin[kb * 128 : (kb + 1) * 128, :], in_=c_kb)

            # ---------------- AllReduce ----------------
            nc.gpsimd.collective_compute(
                "AllReduce",
                mybir.AluOpType.add,
                ins=[arin.opt()],
                outs=[arout.opt()],
                replica_groups=[list(range(n_cores))],
            )

            # ---------------- build mm2 stationaries ----------------
            # AR output is Y*SCALE_M in fp16; use it directly as the
            # stationary (the epilogue descales by SCALE_M*SCALE_B).
            ytA, ytB = [], []
            for kb in range(nkb):
                a_t = sbx.tile([128, 128], fp, tag=f"ytA{kb}", name=f"ytA{kb}")
                nc.sync.dma_start(out=a_t, in_=arout[kb * 128 : (kb + 1) * 128, :])
                b_t = sbx.tile([128, 128], fp, tag=f"ytB{kb}", name=f"ytB{kb}")
                nc.vector.tensor_scalar_mul(b_t[:, 0:64], a_t[:, 64:128], -1.0)
                nc.vector.tensor_copy(b_t[:, 64:128], a_t[:, 0:64])
                ytA.append(a_t)
                ytB.append(b_t)

            # ---------------- mm2 ----------------
            for pr in range(ndc // 2):
                dc0, dc1 = 2 * pr, 2 * pr + 1
                s0 = slice(dc0 * 512, (dc0 + 1) * 512)
                s1 = slice(dc1 * 512, (dc1 + 1) * 512)
                sp_pair = slice(dc0 * 512, (dc0 + 2) * 512)
                po0 = ps.tile([128, 512], f32, tag="po0", name="po0")
                po1 = ps.tile([128, 512], f32, tag="po1", name="po1")
                for kb in range(nkb):
                    rs = slice(kb * 128, (kb + 1) * 128)
                    r01 = sb.tile([128, 1024], fp, tag="bnr01", name="bnr01", bufs=8)
                    nc.sync.dma_start(out=r01, in_=bnr[rs, sp_pair])
                    i01 = sb.tile([128, 1024], fp, tag="bni01", name="bni01", bufs=8)
                    nc.sync.dma_start(out=i01, in_=bni[rs, sp_pair])
                    st, sp = kb == 0, kb == nkb - 1
                    nc.tensor.matmul(po0, lhsT=ytA[kb], rhs=r01[:, 0:512], start=st, stop=False)
                    nc.tensor.matmul(po1, lhsT=ytA[kb], rhs=r01[:, 512:1024], start=st, stop=False)
                    nc.tensor.matmul(po0, lhsT=ytB[kb], rhs=i01[:, 0:512], start=False, stop=sp)
                    nc.tensor.matmul(po1, lhsT=ytB[kb], rhs=i01[:, 512:1024], start=False, stop=sp)
                o0 = sb.tile([128, 512], f32, tag="o0", name="o0", bufs=2)
                nc.vector.tensor_scalar_mul(o0, po0, 1.0 / (SCALE_M * SCALE_B))
                nc.sync.dma_start(out=out[:, s0], in_=o0)
                o1 = sb.tile([128, 512], f32, tag="o1", name="o1", bufs=2)
                nc.vector.tensor_scalar_mul(o1, po1, 1.0 / (SCALE_M * SCALE_B))
                nc.sync.dma_start(out=out[:, s1], in_=o1)

    nc.compile()
    return nc


def _get_nc(n_cores=NCORES, k=K, dl=DL):
    key = (n_cores, k, dl)
    if key not in _nc_cache:
        _nc_cache[key] = build_nc(n_cores, k, dl)
    return _nc_cache[key]


def _prep_in_maps(X_re, X_im, bases_re, bases_im, weight_re, weight_im):
    cdt = np.float16 if COMPUTE_DT == "float16" else None
    if cdt is None:
        import ml_dtypes

        cdt = ml_dtypes.bfloat16

    f32 = np.float32
    X_re = np.asarray(X_re, f32)
    X_im = np.asarray(X_im, f32)
    bases_re = np.asarray(bases_re, f32)
    bases_im = np.asarray(bases_im, f32)
    wr = np.asarray(weight_re, f32)[:, None]
    wi = np.asarray(weight_im, f32)[:, None]

    # M = diag(w) @ conj(B): Mr = wr*Br + wi*Bi ; Mi = wi*Br - wr*Bi
    mr = (wr * bases_re + wi * bases_im) * np.float32(SCALE_M)
    mi = (wi * bases_re - wr * bases_im) * np.float32(SCALE_M)
    bsr = bases_re * np.float32(SCALE_B)
    bsi = bases_im * np.float32(SCALE_B)

    in_maps = []
    for c in range(NCORES):
        lo = c * DL
        hi = min((c + 1) * DL, D)
        n = hi - lo
        xt = np.zeros((DL, 128), cdt)
        if n > 0:
            xt[:n, 0:64] = X_re[:, lo:hi].T.astype(cdt)
            xt[:n, 64:128] = X_im[:, lo:hi].T.astype(cdt)
        mtr = np.zeros((DL, K), cdt)
        mti = np.zeros((DL, K), cdt)
        bnr = np.zeros((K, DL), cdt)
        bni = np.zeros((K, DL), cdt)
        if n > 0:
            mtr[:n, :] = mr[:, lo:hi].T.astype(cdt)
            mti[:n, :] = mi[:, lo:hi].T.astype(cdt)
            bnr[:, :n] = bsr[:, lo:hi].astype(cdt)
            bni[:, :n] = bsi[:, lo:hi].astype(cdt)
        in_maps.append({"xt": xt, "mtr": mtr, "mti": mti, "bnr": bnr, "bni": bni})
    return in_maps


def run(inputs, trace=False, trace_kwargs=None):
    """Returns (full complex64 output [64, 32400], BassKernelResults)."""
    from concourse.bass_utils import run_bass_kernel_spmd

    in_maps = _prep_in_maps(**inputs)
    nc = _get_nc()
    res = run_bass_kernel_spmd(
        nc,
        in_maps,
        core_ids=list(range(NCORES)),
        trace=trace,
        **(trace_kwargs or {}),
    )
    parts = []
    for c in range(NCORES):
        o = res.results[c]["out"]
        parts.append(o[0:64, :] + 1j * o[64:128, :].astype(np.complex64))
    full = np.concatenate(parts, axis=1)[:, :D].astype(np.complex64)
    return full, res


def kernel(**inputs) -> np.ndarray:
    out, _ = run(inputs, trace=False)
    return out



# revision 4
# speedup vs baseline: 1.2065x; 1.2065x over previous
"""ComplexDenseSO2 Trainium2 kernel (v2).

Computes out = (X @ conj(B)^T * w) @ B for complex X [64, 32400],
B [2048, 32400], w [2048], given as separate re/im fp32 planes.

Strategy (tensor-parallel over D across 8 cores):
  - Fold w into the first-matmul operand on the host:
    M = diag(w) @ conj(B), so mm1 output IS Y = X @ M^T.
  - Pad D 32400 -> 32768; core c owns d-slice [c*4096, (c+1)*4096).
  - mm1 merges re/im into ONE PSUM accumulator using two stationary
    variants S_A = (Xr|Xi), S_B = (-Xi|Xr): rows 0:64 = Yr, 64:128 = Yi.
    k is processed in 2 halves of 1024 (2 PSUM banks each) so each
    half's AllReduce overlaps the other half's compute.
  - Per half: PSUM -> SBUF f32, PE-transpose to [k, j], cast fp16,
    one 256KB DMA to DRAM, AllReduce(add) over the 8 cores.
  - mm2: k-outer with all 8 PSUM banks holding the full [128, 4096]
    f32 output; stationaries ytA = Y^T, ytB = (-Yi^T|Yr^T) built from
    the AR output; rhs B tiles streamed as 1MB fully-contiguous DMAs.
  - DMA engine split: input streams (X, M, B) issue on nc.sync;
    AR-coupled transfers (arin writes, arout reads) issue on
    nc.scalar so B prefetch is never blocked behind an AR wait.
  - Host pre-lays out every tensor so each big DMA is >=1MB with
    >=8KB contiguous bytes per partition line.
  - fp16 operands use power-of-2 prescales (M*1024, B*256) to stay
    clear of fp16 subnormals; the epilogue descales by 2^-18.
"""

import sys

if "/opt/trn_rl_repo" not in sys.path:
    sys.path.insert(0, "/opt/trn_rl_repo")

import numpy as np

B_, K, D = 64, 2048, 32400
NCORES = 8
DP = 32768
DL = DP // NCORES  # 4096
NDT = DL // 128    # 32 d-tiles
KH = K // 2        # 1024 k per half
NKB = K // 128     # 16 k-blocks
MCH = 8            # d-tiles per M DMA chunk (2MB per component)

SCALE_M = 1024.0
SCALE_B = 256.0

_nc_cache = {}


def build_nc(n_cores=NCORES):
    import concourse.mybir as mybir
    from concourse import bacc
    import concourse.tile as tile
    from concourse.masks import make_identity

    fp = mybir.dt.float16
    f32 = mybir.dt.float32

    nc = bacc.Bacc(
        trn_type="TRN2",
        target_bir_lowering=False,
        debug=False,
        num_devices=n_cores,
    )
    # xa[p, dt*128+j] = S_A[dt*128+p, j]; S_A = (Xr|Xi) along j
    xa = nc.dram_tensor("xa", [128, DL], fp, kind="ExternalInput")
    xb = nc.dram_tensor("xb", [128, DL], fp, kind="ExternalInput")
    # mrh[p, (half*NDT+dt)*KH + kk] = Mr[half*KH+kk, lo+dt*128+p]
    mrh = nc.dram_tensor("mrh", [128, 2 * NDT * KH], fp, kind="ExternalInput")
    mih = nc.dram_tensor("mih", [128, 2 * NDT * KH], fp, kind="ExternalInput")
    # bnr[p, kb*DL + dd] = Br[kb*128+p, lo+dd] * SCALE_B
    bnr = nc.dram_tensor("bnr", [128, NKB * DL], fp, kind="ExternalInput")
    bni = nc.dram_tensor("bni", [128, NKB * DL], fp, kind="ExternalInput")
    out = nc.dram_tensor("out", [128, DL], f32, kind="ExternalOutput")

    with tile.TileContext(nc) as tc:
        with (
            tc.tile_pool(name="sb", bufs=1) as sb,
            tc.tile_pool(name="sbx", bufs=1) as sbx,
            tc.tile_pool(name="ps", bufs=1, space="PSUM") as ps,
            tc.tile_pool(name="dram", bufs=1, space="DRAM") as dram,
        ):
            ident = sbx.tile([128, 128], f32, tag="ident")
            make_identity(nc, ident)

            xa_s = sbx.tile([128, DL], fp, tag="xa_s", name="xa_s")
            nc.sync.dma_start(out=xa_s, in_=xa.ap())
            xb_s = sbx.tile([128, DL], fp, tag="xb_s", name="xb_s")
            nc.sync.dma_start(out=xb_s, in_=xb.ap())

            arin = []
            arout = []
            for h in range(2):
                ai = dram.tile([KH, 128], fp, tag=f"arin{h}", name=f"arin{h}")
                ao = dram.tile(
                    [KH, 128], fp, tag=f"arout{h}", name=f"arout{h}",
                    addr_space="Shared",
                )
                arin.append(ai)
                arout.append(ao)

            # ---------------- mm1: two k-halves ----------------
            for half in range(2):
                acc0 = ps.tile([128, 512], f32, tag="p0", name="acc0", bufs=1)
                acc1 = ps.tile([128, 512], f32, tag="p1", name="acc1", bufs=1)
                accs = [acc0, acc1]
                for g in range(NDT // MCH):
                    base = (half * NDT + g * MCH) * KH
                    mr_t = sb.tile([128, MCH * KH], fp, tag="mr", name="mr", bufs=2)
                    nc.sync.dma_start(out=mr_t, in_=mrh[:, base : base + MCH * KH])
                    mi_t = sb.tile([128, MCH * KH], fp, tag="mi", name="mi", bufs=2)
                    nc.sync.dma_start(out=mi_t, in_=mih[:, base : base + MCH * KH])
                    for t in range(MCH):
                        dt = g * MCH + t
                        st, sp = dt == 0, dt == NDT - 1
                        xs_a = xa_s[:, dt * 128 : (dt + 1) * 128]
                        xs_b = xb_s[:, dt * 128 : (dt + 1) * 128]
                        ts = t * KH
                        nc.tensor.matmul(acc0, lhsT=xs_a, rhs=mr_t[:, ts : ts + 512], start=st, stop=False)
                        nc.tensor.matmul(acc1, lhsT=xs_a, rhs=mr_t[:, ts + 512 : ts + 1024], start=st, stop=False)
                        nc.tensor.matmul(acc0, lhsT=xs_b, rhs=mi_t[:, ts : ts + 512], start=False, stop=sp)
                        nc.tensor.matmul(acc1, lhsT=xs_b, rhs=mi_t[:, ts + 512 : ts + 1024], start=False, stop=sp)

                # Evacuate Y[j, k-half], transpose to [k, j], ship to AR.
                pscr = sb.tile([128, KH], f32, tag="pscr", name="pscr", bufs=2)
                nc.vector.tensor_copy(pscr[:, 0:512], accs[0])
                nc.vector.tensor_copy(pscr[:, 512:1024], accs[1])
                yt_sb = sb.tile([128, KH // 128 * 128], fp, tag="yt_sb", name="yt_sb", bufs=2)
                for b in range(KH // 128):
                    tp = ps.tile([128, 128], f32, tag=f"p{2 + (b % 2)}", name="tp", bufs=1)
                    nc.tensor.transpose(tp, pscr[:, b * 128 : (b + 1) * 128], ident)
                    nc.vector.tensor_copy(yt_sb[:, b * 128 : (b + 1) * 128], tp)
                nc.scalar.dma_start(
                    out=arin[half].rearrange("(t p) j -> p t j", p=128),
                    in_=yt_sb.rearrange("p (t j) -> p t j", j=128),
                )
                nc.gpsimd.collective_compute(
                    "AllReduce",
                    mybir.AluOpType.add,
                    ins=[arin[half].opt()],
                    outs=[arout[half].opt()],
                    replica_groups=[list(range(n_cores))],
                )

            # ---------------- mm2 stationaries from AR output ----------------
            ytA = []
            ytB = []
            for half in range(2):
                a_t = sbx.tile([128, KH], fp, tag=f"ytA{half}", name=f"ytA{half}")
                nc.scalar.dma_start(
                    out=a_t.rearrange("p (t j) -> p t j", j=128),
                    in_=arout[half].rearrange("(t p) j -> p t j", p=128),
                )
                b_t = sbx.tile([128, KH], fp, tag=f"ytB{half}", name=f"ytB{half}")
                for b in range(KH // 128):
                    o = b * 128
                    nc.vector.tensor_scalar_mul(b_t[:, o : o + 64], a_t[:, o + 64 : o + 128], -1.0)
                    nc.vector.tensor_copy(b_t[:, o + 64 : o + 128], a_t[:, o : o + 64])
                ytA.append(a_t)
                ytB.append(b_t)

            # ---------------- mm2: k-outer, 8 PSUM banks ----------------
            pos = []
            for i in range(8):
                po = ps.tile([128, 512], f32, tag=f"p{i}", name=f"po{i}", bufs=1)
                pos.append(po)
            for kb in range(NKB):
                half, b = kb // (KH // 128), kb % (KH // 128)
                br_t = sb.tile([128, DL], fp, tag="br", name="br", bufs=3)
                nc.sync.dma_start(out=br_t, in_=bnr[:, kb * DL : (kb + 1) * DL])
                bi_t = sb.tile([128, DL], fp, tag="bi", name="bi", bufs=3)
                nc.sync.dma_start(out=bi_t, in_=bni[:, kb * DL : (kb + 1) * DL])
                ya = ytA[half][:, b * 128 : (b + 1) * 128]
                yb = ytB[half][:, b * 128 : (b + 1) * 128]
                st, sp = kb == 0, kb == NKB - 1
                for i in range(8):
                    nc.tensor.matmul(pos[i], lhsT=ya, rhs=br_t[:, i * 512 : (i + 1) * 512], start=st, stop=False)
                for i in range(8):
                    nc.tensor.matmul(pos[i], lhsT=yb, rhs=bi_t[:, i * 512 : (i + 1) * 512], start=False, stop=sp)

            osb = sb.tile([128, DL], f32, tag="osb", name="osb", bufs=1)
            for i in range(8):
                nc.vector.tensor_scalar_mul(
                    osb[:, i * 512 : (i + 1) * 512], pos[i], 1.0 / (SCALE_M * SCALE_B)
                )
            nc.sync.dma_start(out=out.ap(), in_=osb)

    nc.compile()
    return nc


def _get_nc(n_cores=NCORES):
    if n_cores not in _nc_cache:
        _nc_cache[n_cores] = build_nc(n_cores)
    return _nc_cache[n_cores]


def _prep_in_maps(X_re, X_im, bases_re, bases_im, weight_re, weight_im):
    cdt = np.float16
    f32 = np.float32
    X_re = np.asarray(X_re, f32)
    X_im = np.asarray(X_im, f32)
    bases_re = np.asarray(bases_re, f32)
    bases_im = np.asarray(bases_im, f32)
    wr = np.asarray(weight_re, f32)[:, None]
    wi = np.asarray(weight_im, f32)[:, None]

    # M = diag(w) @ conj(B): Mr = wr*Br + wi*Bi ; Mi = wi*Br - wr*Bi
    mr = (wr * bases_re + wi * bases_im) * f32(SCALE_M)
    mi = (wi * bases_re - wr * bases_im) * f32(SCALE_M)
    bsr = bases_re * f32(SCALE_B)
    bsi = bases_im * f32(SCALE_B)

    in_maps = []
    for c in range(NCORES):
        lo = c * DL
        hi = min((c + 1) * DL, D)
        n = hi - lo

        xat = np.zeros((DL, 128), f32)
        xbt = np.zeros((DL, 128), f32)
        if n > 0:
            xat[:n, 0:64] = X_re[:, lo:hi].T
            xat[:n, 64:128] = X_im[:, lo:hi].T
            xbt[:n, 0:64] = -X_im[:, lo:hi].T
            xbt[:n, 64:128] = X_re[:, lo:hi].T
        # [DL,128] -> [128, DL] with xa[p, dt*128+j] = xat[dt*128+p, j]
        xa = xat.reshape(NDT, 128, 128).transpose(1, 0, 2).reshape(128, DL).astype(cdt)
        xb = xbt.reshape(NDT, 128, 128).transpose(1, 0, 2).reshape(128, DL).astype(cdt)

        def m_layout(m):
            mp = np.zeros((K, DL), f32)
            if n > 0:
                mp[:, :n] = m[:, lo:hi]
            # mrh[p, (half*NDT+dt)*KH + kk] = mp[half*KH+kk, dt*128+p]
            t = mp.reshape(2, KH, NDT, 128)
            return t.transpose(3, 0, 2, 1).reshape(128, 2 * NDT * KH).astype(cdt)

        def b_layout(bm):
            bp = np.zeros((K, DL), f32)
            if n > 0:
                bp[:, :n] = bm[:, lo:hi]
            # bnr[p, kb*DL + dd] = bp[kb*128+p, dd]
            t = bp.reshape(NKB, 128, DL)
            return t.transpose(1, 0, 2).reshape(128, NKB * DL).astype(cdt)

        in_maps.append({
            "xa": xa,
            "xb": xb,
            "mrh": m_layout(mr),
            "mih": m_layout(mi),
            "bnr": b_layout(bsr),
            "bni": b_layout(bsi),
        })
    return in_maps


def run(inputs, trace=False, trace_kwargs=None):
    """Returns (full complex64 output [64, 32400], BassKernelResults)."""
    from concourse.bass_utils import run_bass_kernel_spmd

    in_maps = _prep_in_maps(**inputs)
    nc = _get_nc()
    res = run_bass_kernel_spmd(
        nc,
        in_maps,
        core_ids=list(range(NCORES)),
        trace=trace,
        **(trace_kwargs or {}),
    )
    parts = []
    for c in range(NCORES):
        o = res.results[c]["out"]
        parts.append(o[0:64, :] + 1j * o[64:128, :].astype(np.complex64))
    full = np.concatenate(parts, axis=1)[:, :D].astype(np.complex64)
    return full, res


def kernel(**inputs) -> np.ndarray:
    out, _ = run(inputs, trace=False)
    return out


# revision 7
# speedup vs baseline: 1.2153x; 1.0073x over previous
"""ComplexDenseSO2 Trainium2 kernel (v3).

Computes out = (X @ conj(B)^T * w) @ B for complex X [64, 32400],
B [2048, 32400], w [2048], given as separate re/im fp32 planes.

Strategy (tensor-parallel over D across 8 cores):
  - Fold w into the first-matmul operand on the host:
    M = diag(w) @ conj(B), so mm1 output IS Y = X @ M^T.
  - Pad D 32400 -> 32768; core c owns d-slice [c*4096, (c+1)*4096).
  - mm1 merges re/im into ONE PSUM accumulator using two stationary
    variants S_A = (Xr|Xi), S_B = (-Xi|Xr): rows 0:64 = Yr, 64:128 = Yi.
    k is processed in 4 QUARTERS of 512 (1 PSUM bank each) so the four
    128KB AllReduces trigger early and pipeline behind each other while
    mm1/mm2 compute continues.
  - Per quarter: PSUM -> SBUF f32, PE-transpose to [k, j], cast fp16,
    one 128KB DMA to DRAM, AllReduce(add) over the 8 cores.
  - mm2: k-outer with all 8 PSUM banks holding the full [128, 4096]
    f32 output; stationaries ytA = Y^T, ytB = (-Yi^T|Yr^T) built from
    the AR outputs; rhs B tiles streamed as 1MB fully-contiguous DMAs.
  - DMA engine split: input streams (X, M, B) issue on nc.sync;
    AR-coupled transfers (arin writes, arout reads) issue on
    nc.scalar with explicit ordering deps so the scheduler cannot
    park an AR-output read in front of a later AR-input write.
  - Host pre-lays out every tensor so each big DMA is >=1MB with
    >=8KB contiguous bytes per partition line.
  - fp16 operands use power-of-2 prescales (M*1024, B*256) to stay
    clear of fp16 subnormals; the epilogue descales by 2^-18.
"""

import sys

if "/opt/trn_rl_repo" not in sys.path:
    sys.path.insert(0, "/opt/trn_rl_repo")

import numpy as np

B_, K, D = 64, 2048, 32400
NCORES = 8
DP = 32768
DL = DP // NCORES  # 4096
NDT = DL // 128    # 32 d-tiles
NQ = 4             # k-quarters
KW = K // NQ       # 512 k per quarter
NKB = K // 128     # 16 k-blocks
MCH = 8            # d-tiles per M DMA chunk (1MB per component)

SCALE_M = 1024.0
SCALE_B = 256.0

_nc_cache = {}


def build_nc(n_cores=NCORES):
    import concourse.mybir as mybir
    from concourse import bacc
    import concourse.tile as tile
    from concourse.masks import make_identity
    from concourse.tile_rust import add_dep_helper

    fp = mybir.dt.float16
    f32 = mybir.dt.float32

    nc = bacc.Bacc(
        trn_type="TRN2",
        target_bir_lowering=False,
        debug=False,
        num_devices=n_cores,
    )
    # xa[p, dt*128+j] = S_A[dt*128+p, j]; S_A = (Xr|Xi) along j
    xa = nc.dram_tensor("xa", [128, DL], fp, kind="ExternalInput")
    xb = nc.dram_tensor("xb", [128, DL], fp, kind="ExternalInput")
    # mrh[p, (q*NDT+dt)*KW + kk] = Mr[q*KW+kk, lo+dt*128+p]
    mrh = nc.dram_tensor("mrh", [128, NQ * NDT * KW], fp, kind="ExternalInput")
    mih = nc.dram_tensor("mih", [128, NQ * NDT * KW], fp, kind="ExternalInput")
    # bnr[p, kb*DL + dd] = Br[kb*128+p, lo+dd] * SCALE_B
    bnr = nc.dram_tensor("bnr", [128, NKB * DL], fp, kind="ExternalInput")
    bni = nc.dram_tensor("bni", [128, NKB * DL], fp, kind="ExternalInput")
    out = nc.dram_tensor("out", [128, DL], f32, kind="ExternalOutput")

    with tile.TileContext(nc) as tc:
        with (
            tc.tile_pool(name="sb", bufs=1) as sb,
            tc.tile_pool(name="sbx", bufs=1) as sbx,
            tc.tile_pool(name="ps", bufs=1, space="PSUM") as ps,
            tc.tile_pool(name="dram", bufs=1, space="DRAM") as dram,
        ):
            ident = sbx.tile([128, 128], f32, tag="ident")
            make_identity(nc, ident)

            xa_s = sbx.tile([128, DL], fp, tag="xa_s", name="xa_s")
            nc.sync.dma_start(out=xa_s, in_=xa.ap())
            xb_s = sbx.tile([128, DL], fp, tag="xb_s", name="xb_s")
            nc.sync.dma_start(out=xb_s, in_=xb.ap())

            arin = []
            arout = []
            for q in range(NQ):
                ai = dram.tile([KW, 128], fp, tag=f"arin{q}", name=f"arin{q}")
                ao = dram.tile(
                    [KW, 128], fp, tag=f"arout{q}", name=f"arout{q}",
                    addr_space="Shared",
                )
                arin.append(ai)
                arout.append(ao)

            # ---------------- mm1: four k-quarters ----------------
            scalar_dmas = []  # for explicit ordering on the scalar queue
            for q in range(NQ):
                acc = ps.tile([128, KW], f32, tag=f"p{q % 2}", name="acc", bufs=1)
                for g in range(NDT // MCH):
                    base = (q * NDT + g * MCH) * KW
                    mr_t = sb.tile([128, MCH * KW], fp, tag="mr", name="mr", bufs=3)
                    nc.sync.dma_start(out=mr_t, in_=mrh[:, base : base + MCH * KW])
                    mi_t = sb.tile([128, MCH * KW], fp, tag="mi", name="mi", bufs=3)
                    nc.sync.dma_start(out=mi_t, in_=mih[:, base : base + MCH * KW])
                    for t in range(MCH):
                        dt = g * MCH + t
                        st, sp = dt == 0, dt == NDT - 1
                        xs_a = xa_s[:, dt * 128 : (dt + 1) * 128]
                        xs_b = xb_s[:, dt * 128 : (dt + 1) * 128]
                        ts = t * KW
                        nc.tensor.matmul(acc, lhsT=xs_a, rhs=mr_t[:, ts : ts + KW], start=st, stop=False)
                        nc.tensor.matmul(acc, lhsT=xs_b, rhs=mi_t[:, ts : ts + KW], start=False, stop=sp)

                # Evacuate Y[j, quarter], transpose to [k, j], ship to AR.
                pscr = sb.tile([128, KW], f32, tag="pscr", name="pscr", bufs=2)
                nc.vector.tensor_copy(pscr, acc)
                yt_sb = sb.tile([128, KW], fp, tag="yt_sb", name="yt_sb", bufs=2)
                for b in range(KW // 128):
                    tp = ps.tile([128, 128], f32, tag=f"p{4 + (b % 2)}", name="tp", bufs=1)
                    nc.tensor.transpose(tp, pscr[:, b * 128 : (b + 1) * 128], ident)
                    nc.vector.tensor_copy(yt_sb[:, b * 128 : (b + 1) * 128], tp)
                w_inst = nc.scalar.dma_start(
                    out=arin[q].rearrange("(t p) j -> p t j", p=128),
                    in_=yt_sb.rearrange("p (t j) -> p t j", j=128),
                )
                scalar_dmas.append(w_inst)
                nc.gpsimd.collective_compute(
                    "AllReduce",
                    mybir.AluOpType.add,
                    ins=[arin[q].opt()],
                    outs=[arout[q].opt()],
                    replica_groups=[list(range(n_cores))],
                )

            # ---------------- mm2 stationaries from AR outputs ----------------
            ytA = []
            ytB = []
            for q in range(NQ):
                a_t = sbx.tile([128, KW], fp, tag=f"ytA{q}", name=f"ytA{q}")
                r_inst = nc.scalar.dma_start(
                    out=a_t.rearrange("p (t j) -> p t j", j=128),
                    in_=arout[q].rearrange("(t p) j -> p t j", p=128),
                )
                scalar_dmas.append(r_inst)
                b_t = sbx.tile([128, KW], fp, tag=f"ytB{q}", name=f"ytB{q}")
                for b in range(KW // 128):
                    o = b * 128
                    nc.vector.tensor_scalar_mul(b_t[:, o : o + 64], a_t[:, o + 64 : o + 128], -1.0)
                    nc.vector.tensor_copy(b_t[:, o + 64 : o + 128], a_t[:, o : o + 64])
                ytA.append(a_t)
                ytB.append(b_t)

            # Pin the scalar-queue order: every arout read sits after every
            # arin write, and the queue follows emission order. Without this
            # the scheduler can park an AR-output read (waiting on AR k) in
            # front of a later AR-input write, stalling the next AR.
            from_i = None
            for inst in scalar_dmas:
                if from_i is not None:
                    add_dep_helper(inst.ins, from_i.ins, sync=False,
                                   reason="scalar DMA queue order")
                from_i = inst

            # ---------------- mm2: k-outer, 8 PSUM banks ----------------
            pos = []
            for i in range(8):
                po = ps.tile([128, 512], f32, tag=f"p{i}", name=f"po{i}", bufs=1)
                pos.append(po)
            for kb in range(NKB):
                q, b = kb // (KW // 128), kb % (KW // 128)
                br_t = sb.tile([128, DL], fp, tag="br", name="br", bufs=3)
                nc.sync.dma_start(out=br_t, in_=bnr[:, kb * DL : (kb + 1) * DL])
                bi_t = sb.tile([128, DL], fp, tag="bi", name="bi", bufs=3)
                nc.sync.dma_start(out=bi_t, in_=bni[:, kb * DL : (kb + 1) * DL])
                ya = ytA[q][:, b * 128 : (b + 1) * 128]
                yb = ytB[q][:, b * 128 : (b + 1) * 128]
                st, sp = kb == 0, kb == NKB - 1
                for i in range(8):
                    nc.tensor.matmul(pos[i], lhsT=ya, rhs=br_t[:, i * 512 : (i + 1) * 512], start=st, stop=False)
                for i in range(8):
                    nc.tensor.matmul(pos[i], lhsT=yb, rhs=bi_t[:, i * 512 : (i + 1) * 512], start=False, stop=sp)

            # Descale by 1/(SCALE_M*SCALE_B) happens on the host during
            # output assembly; here only evacuate PSUM -> SBUF.
            osb = sb.tile([128, DL], f32, tag="osb", name="osb", bufs=1)
            for i in range(8):
                if i % 2 == 0:
                    nc.vector.tensor_copy(osb[:, i * 512 : (i + 1) * 512], pos[i])
                else:
                    nc.scalar.copy(osb[:, i * 512 : (i + 1) * 512], pos[i])
            nc.sync.dma_start(out=out[:, 0 : DL // 2], in_=osb[:, 0 : DL // 2])
            nc.sync.dma_start(out=out[:, DL // 2 : DL], in_=osb[:, DL // 2 : DL])

    nc.compile()
    return nc


def _get_nc(n_cores=NCORES):
    if n_cores not in _nc_cache:
        _nc_cache[n_cores] = build_nc(n_cores)
    return _nc_cache[n_cores]


def _prep_in_maps(X_re, X_im, bases_re, bases_im, weight_re, weight_im):
    cdt = np.float16
    f32 = np.float32
    X_re = np.asarray(X_re, f32)
    X_im = np.asarray(X_im, f32)
    bases_re = np.asarray(bases_re, f32)
    bases_im = np.asarray(bases_im, f32)
    wr = np.asarray(weight_re, f32)[:, None]
    wi = np.asarray(weight_im, f32)[:, None]

    # M = diag(w) @ conj(B): Mr = wr*Br + wi*Bi ; Mi = wi*Br - wr*Bi
    mr = (wr * bases_re + wi * bases_im) * f32(SCALE_M)
    mi = (wi * bases_re - wr * bases_im) * f32(SCALE_M)
    bsr = bases_re * f32(SCALE_B)
    bsi = bases_im * f32(SCALE_B)

    in_maps = []
    for c in range(NCORES):
        lo = c * DL
        hi = min((c + 1) * DL, D)
        n = hi - lo

        xat = np.zeros((DL, 128), f32)
        xbt = np.zeros((DL, 128), f32)
        if n > 0:
            xat[:n, 0:64] = X_re[:, lo:hi].T
            xat[:n, 64:128] = X_im[:, lo:hi].T
            xbt[:n, 0:64] = -X_im[:, lo:hi].T
            xbt[:n, 64:128] = X_re[:, lo:hi].T
        # [DL,128] -> [128, DL] with xa[p, dt*128+j] = xat[dt*128+p, j]
        xa = xat.reshape(NDT, 128, 128).transpose(1, 0, 2).reshape(128, DL).astype(cdt)
        xb = xbt.reshape(NDT, 128, 128).transpose(1, 0, 2).reshape(128, DL).astype(cdt)

        def m_layout(m):
            mp = np.zeros((K, DL), f32)
            if n > 0:
                mp[:, :n] = m[:, lo:hi]
            # mrh[p, (q*NDT+dt)*KW + kk] = mp[q*KW+kk, dt*128+p]
            t = mp.reshape(NQ, KW, NDT, 128)
            return t.transpose(3, 0, 2, 1).reshape(128, NQ * NDT * KW).astype(cdt)

        def b_layout(bm):
            bp = np.zeros((K, DL), f32)
            if n > 0:
                bp[:, :n] = bm[:, lo:hi]
            # bnr[p, kb*DL + dd] = bp[kb*128+p, dd]
            t = bp.reshape(NKB, 128, DL)
            return t.transpose(1, 0, 2).reshape(128, NKB * DL).astype(cdt)

        in_maps.append({
            "xa": xa,
            "xb": xb,
            "mrh": m_layout(mr),
            "mih": m_layout(mi),
            "bnr": b_layout(bsr),
            "bni": b_layout(bsi),
        })
    return in_maps


def run(inputs, trace=False, trace_kwargs=None):
    """Returns (full complex64 output [64, 32400], BassKernelResults)."""
    from concourse.bass_utils import run_bass_kernel_spmd

    in_maps = _prep_in_maps(**inputs)
    nc = _get_nc()
    res = run_bass_kernel_spmd(
        nc,
        in_maps,
        core_ids=list(range(NCORES)),
        trace=trace,
        **(trace_kwargs or {}),
    )
    dsc = np.float32(1.0 / (SCALE_M * SCALE_B))
    parts = []
    for c in range(NCORES):
        o = res.results[c]["out"]
        parts.append(o[0:64, :] + 1j * o[64:128, :].astype(np.complex64))
    full = (np.concatenate(parts, axis=1)[:, :D] * dsc).astype(np.complex64)
    return full, res


def kernel(**inputs) -> np.ndarray:
    out, _ = run(inputs, trace=False)
    return out


# revision 8
# speedup vs baseline: 1.2418x; 1.0218x over previous
"""ComplexDenseSO2 Trainium2 kernel (v3).

Computes out = (X @ conj(B)^T * w) @ B for complex X [64, 32400],
B [2048, 32400], w [2048], given as separate re/im fp32 planes.

Strategy (tensor-parallel over D across 8 cores):
  - Fold w into the first-matmul operand on the host:
    M = diag(w) @ conj(B), so mm1 output IS Y = X @ M^T.
  - Pad D 32400 -> 32768; core c owns d-slice [c*4096, (c+1)*4096).
  - mm1 merges re/im into ONE PSUM accumulator using two stationary
    variants S_A = (Xr|Xi), S_B = (-Xi|Xr): rows 0:64 = Yr, 64:128 = Yi.
    k is processed in 4 QUARTERS of 512 (1 PSUM bank each) so the four
    128KB AllReduces trigger early and pipeline behind each other while
    mm1/mm2 compute continues.
  - Per quarter: PSUM -> SBUF f32, PE-transpose to [k, j], cast fp16,
    one 128KB DMA to DRAM, AllReduce(add) over the 8 cores.
  - mm2: k-outer with all 8 PSUM banks holding the full [128, 4096]
    f32 output; stationaries ytA = Y^T, ytB = (-Yi^T|Yr^T) built from
    the AR outputs; rhs B tiles streamed as 1MB fully-contiguous DMAs.
  - DMA engine split: input streams (X, M, B) issue on nc.sync;
    AR-coupled transfers (arin writes, arout reads) issue on
    nc.scalar with explicit ordering deps so the scheduler cannot
    park an AR-output read in front of a later AR-input write.
  - Host pre-lays out every tensor so each big DMA is >=1MB with
    >=8KB contiguous bytes per partition line.
  - fp16 operands use power-of-2 prescales (M*1024, B*256) to stay
    clear of fp16 subnormals; the epilogue descales by 2^-18.
"""

import sys

if "/opt/trn_rl_repo" not in sys.path:
    sys.path.insert(0, "/opt/trn_rl_repo")

import numpy as np

B_, K, D = 64, 2048, 32400
NCORES = 8
DP = 32768
DL = DP // NCORES  # 4096
NDT = DL // 128    # 32 d-tiles
NQ = 2             # k-halves
KW = K // NQ       # 1024 k per half
NKB = K // 128     # 16 k-blocks
MCH = 8            # d-tiles per M DMA chunk (2MB per component)

SCALE_M = 1024.0
SCALE_B = 256.0

_nc_cache = {}


def build_nc(n_cores=NCORES):
    import concourse.mybir as mybir
    from concourse import bacc
    import concourse.tile as tile
    from concourse.masks import make_identity
    from concourse.tile_rust import add_dep_helper

    fp = mybir.dt.float16
    f32 = mybir.dt.float32

    nc = bacc.Bacc(
        trn_type="TRN2",
        target_bir_lowering=False,
        debug=False,
        num_devices=n_cores,
    )
    # xa[p, dt*128+j] = S_A[dt*128+p, j]; S_A = (Xr|Xi) along j
    xa = nc.dram_tensor("xa", [128, DL], fp, kind="ExternalInput")
    xb = nc.dram_tensor("xb", [128, DL], fp, kind="ExternalInput")
    # mrh[p, (q*NDT+dt)*KW + kk] = Mr[q*KW+kk, lo+dt*128+p]
    mrh = nc.dram_tensor("mrh", [128, NQ * NDT * KW], fp, kind="ExternalInput")
    mih = nc.dram_tensor("mih", [128, NQ * NDT * KW], fp, kind="ExternalInput")
    # bnr[p, kb*DL + dd] = Br[kb*128+p, lo+dd] * SCALE_B
    bnr = nc.dram_tensor("bnr", [128, NKB * DL], fp, kind="ExternalInput")
    bni = nc.dram_tensor("bni", [128, NKB * DL], fp, kind="ExternalInput")
    out = nc.dram_tensor("out", [128, DL], f32, kind="ExternalOutput")

    with tile.TileContext(nc) as tc:
        with (
            tc.tile_pool(name="sb", bufs=1) as sb,
            tc.tile_pool(name="sbx", bufs=1) as sbx,
            tc.tile_pool(name="ps", bufs=1, space="PSUM") as ps,
            tc.tile_pool(name="dram", bufs=1, space="DRAM") as dram,
        ):
            ident = sbx.tile([128, 128], f32, tag="ident")
            make_identity(nc, ident)

            xa_s = sbx.tile([128, DL], fp, tag="xa_s", name="xa_s")
            nc.sync.dma_start(out=xa_s, in_=xa.ap())
            xb_s = sbx.tile([128, DL], fp, tag="xb_s", name="xb_s")
            nc.sync.dma_start(out=xb_s, in_=xb.ap())

            arin = []
            arout = []
            for q in range(NQ):
                ai = dram.tile([KW, 128], fp, tag=f"arin{q}", name=f"arin{q}")
                ao = dram.tile(
                    [KW, 128], fp, tag=f"arout{q}", name=f"arout{q}",
                    addr_space="Shared",
                )
                arin.append(ai)
                arout.append(ao)

            # ---------------- mm1: four k-quarters ----------------
            scalar_dmas = []  # for explicit ordering on the scalar queue
            for q in range(NQ):
                acc0 = ps.tile([128, 512], f32, tag=f"p{2 * q}", name="acc0", bufs=1)
                acc1 = ps.tile([128, 512], f32, tag=f"p{2 * q + 1}", name="acc1", bufs=1)
                for g in range(NDT // MCH):
                    base = (q * NDT + g * MCH) * KW
                    mr_t = sb.tile([128, MCH * KW], fp, tag="mr", name="mr", bufs=2)
                    nc.sync.dma_start(out=mr_t, in_=mrh[:, base : base + MCH * KW])
                    mi_t = sb.tile([128, MCH * KW], fp, tag="mi", name="mi", bufs=2)
                    nc.sync.dma_start(out=mi_t, in_=mih[:, base : base + MCH * KW])
                    for t in range(MCH):
                        dt = g * MCH + t
                        st, sp = dt == 0, dt == NDT - 1
                        xs_a = xa_s[:, dt * 128 : (dt + 1) * 128]
                        xs_b = xb_s[:, dt * 128 : (dt + 1) * 128]
                        ts = t * KW
                        nc.tensor.matmul(acc0, lhsT=xs_a, rhs=mr_t[:, ts : ts + 512], start=st, stop=False)
                        nc.tensor.matmul(acc1, lhsT=xs_a, rhs=mr_t[:, ts + 512 : ts + 1024], start=st, stop=False)
                        nc.tensor.matmul(acc0, lhsT=xs_b, rhs=mi_t[:, ts : ts + 512], start=False, stop=sp)
                        nc.tensor.matmul(acc1, lhsT=xs_b, rhs=mi_t[:, ts + 512 : ts + 1024], start=False, stop=sp)

                # Evacuate Y[j, half], transpose to [k, j], ship to AR.
                pscr = sb.tile([128, KW], f32, tag="pscr", name="pscr", bufs=2)
                nc.vector.tensor_copy(pscr[:, 0:512], acc0)
                nc.scalar.copy(pscr[:, 512:1024], acc1)
                yt_sb = sb.tile([128, KW], fp, tag="yt_sb", name="yt_sb", bufs=2)
                for b in range(KW // 128):
                    tp = ps.tile([128, 128], f32, tag=f"p{4 + (b % 2)}", name="tp", bufs=1)
                    nc.tensor.transpose(tp, pscr[:, b * 128 : (b + 1) * 128], ident)
                    nc.vector.tensor_copy(yt_sb[:, b * 128 : (b + 1) * 128], tp)
                w_inst = nc.scalar.dma_start(
                    out=arin[q].rearrange("(t p) j -> p t j", p=128),
                    in_=yt_sb.rearrange("p (t j) -> p t j", j=128),
                )
                scalar_dmas.append(w_inst)
                nc.gpsimd.collective_compute(
                    "AllReduce",
                    mybir.AluOpType.add,
                    ins=[arin[q].opt()],
                    outs=[arout[q].opt()],
                    replica_groups=[list(range(n_cores))],
                )

            # ---------------- mm2 stationaries from AR outputs ----------------
            ytA = []
            ytB = []
            for q in range(NQ):
                a_t = sbx.tile([128, KW], fp, tag=f"ytA{q}", name=f"ytA{q}")
                r_inst = nc.scalar.dma_start(
                    out=a_t.rearrange("p (t j) -> p t j", j=128),
                    in_=arout[q].rearrange("(t p) j -> p t j", p=128),
                )
                scalar_dmas.append(r_inst)
                b_t = sbx.tile([128, KW], fp, tag=f"ytB{q}", name=f"ytB{q}")
                for b in range(KW // 128):
                    o = b * 128
                    nc.vector.tensor_scalar_mul(b_t[:, o : o + 64], a_t[:, o + 64 : o + 128], -1.0)
                    nc.vector.tensor_copy(b_t[:, o + 64 : o + 128], a_t[:, o : o + 64])
                ytA.append(a_t)
                ytB.append(b_t)

            # Pin the scalar-queue order: every arout read sits after every
            # arin write, and the queue follows emission order. Without this
            # the scheduler can park an AR-output read (waiting on AR k) in
            # front of a later AR-input write, stalling the next AR.
            from_i = None
            for inst in scalar_dmas:
                if from_i is not None:
                    add_dep_helper(inst.ins, from_i.ins, sync=False,
                                   reason="scalar DMA queue order")
                from_i = inst

            # ---------------- mm2: k-outer, 8 PSUM banks ----------------
            pos = []
            for i in range(8):
                po = ps.tile([128, 512], f32, tag=f"p{i}", name=f"po{i}", bufs=1)
                pos.append(po)
            for kp in range(NKB // 2):
                br_t = sb.tile([128, 2 * DL], fp, tag="br", name="br", bufs=2)
                nc.sync.dma_start(out=br_t, in_=bnr[:, 2 * kp * DL : (2 * kp + 2) * DL])
                bi_t = sb.tile([128, 2 * DL], fp, tag="bi", name="bi", bufs=2)
                nc.sync.dma_start(out=bi_t, in_=bni[:, 2 * kp * DL : (2 * kp + 2) * DL])
                for j in range(2):
                    kb = 2 * kp + j
                    q, b = kb // (KW // 128), kb % (KW // 128)
                    ya = ytA[q][:, b * 128 : (b + 1) * 128]
                    yb = ytB[q][:, b * 128 : (b + 1) * 128]
                    st, sp = kb == 0, kb == NKB - 1
                    jo = j * DL
                    for i in range(8):
                        nc.tensor.matmul(pos[i], lhsT=ya, rhs=br_t[:, jo + i * 512 : jo + (i + 1) * 512], start=st, stop=False)
                    for i in range(8):
                        nc.tensor.matmul(pos[i], lhsT=yb, rhs=bi_t[:, jo + i * 512 : jo + (i + 1) * 512], start=False, stop=sp)

            # Descale by 1/(SCALE_M*SCALE_B) happens on the host during
            # output assembly; here only evacuate PSUM -> SBUF.
            osb = sb.tile([128, DL], f32, tag="osb", name="osb", bufs=1)
            for i in range(8):
                if i % 2 == 0:
                    nc.vector.tensor_copy(osb[:, i * 512 : (i + 1) * 512], pos[i])
                else:
                    nc.scalar.copy(osb[:, i * 512 : (i + 1) * 512], pos[i])
            nc.sync.dma_start(out=out[:, 0 : DL // 2], in_=osb[:, 0 : DL // 2])
            nc.sync.dma_start(out=out[:, DL // 2 : DL], in_=osb[:, DL // 2 : DL])

    nc.compile()
    return nc


def _get_nc(n_cores=NCORES):
    if n_cores not in _nc_cache:
        _nc_cache[n_cores] = build_nc(n_cores)
    return _nc_cache[n_cores]


def _prep_in_maps(X_re, X_im, bases_re, bases_im, weight_re, weight_im):
    cdt = np.float16
    f32 = np.float32
    X_re = np.asarray(X_re, f32)
    X_im = np.asarray(X_im, f32)
    bases_re = np.asarray(bases_re, f32)
    bases_im = np.asarray(bases_im, f32)
    wr = np.asarray(weight_re, f32)[:, None]
    wi = np.asarray(weight_im, f32)[:, None]

    # M = diag(w) @ conj(B): Mr = wr*Br + wi*Bi ; Mi = wi*Br - wr*Bi
    mr = (wr * bases_re + wi * bases_im) * f32(SCALE_M)
    mi = (wi * bases_re - wr * bases_im) * f32(SCALE_M)
    bsr = bases_re * f32(SCALE_B)
    bsi = bases_im * f32(SCALE_B)

    in_maps = []
    for c in range(NCORES):
        lo = c * DL
        hi = min((c + 1) * DL, D)
        n = hi - lo

        xat = np.zeros((DL, 128), f32)
        xbt = np.zeros((DL, 128), f32)
        if n > 0:
            xat[:n, 0:64] = X_re[:, lo:hi].T
            xat[:n, 64:128] = X_im[:, lo:hi].T
            xbt[:n, 0:64] = -X_im[:, lo:hi].T
            xbt[:n, 64:128] = X_re[:, lo:hi].T
        # [DL,128] -> [128, DL] with xa[p, dt*128+j] = xat[dt*128+p, j]
        xa = xat.reshape(NDT, 128, 128).transpose(1, 0, 2).reshape(128, DL).astype(cdt)
        xb = xbt.reshape(NDT, 128, 128).transpose(1, 0, 2).reshape(128, DL).astype(cdt)

        def m_layout(m):
            mp = np.zeros((K, DL), f32)
            if n > 0:
                mp[:, :n] = m[:, lo:hi]
            # mrh[p, (q*NDT+dt)*KW + kk] = mp[q*KW+kk, dt*128+p]
            t = mp.reshape(NQ, KW, NDT, 128)
            return t.transpose(3, 0, 2, 1).reshape(128, NQ * NDT * KW).astype(cdt)

        def b_layout(bm):
            bp = np.zeros((K, DL), f32)
            if n > 0:
                bp[:, :n] = bm[:, lo:hi]
            # bnr[p, kb*DL + dd] = bp[kb*128+p, dd]
            t = bp.reshape(NKB, 128, DL)
            return t.transpose(1, 0, 2).reshape(128, NKB * DL).astype(cdt)

        in_maps.append({
            "xa": xa,
            "xb": xb,
            "mrh": m_layout(mr),
            "mih": m_layout(mi),
            "bnr": b_layout(bsr),
            "bni": b_layout(bsi),
        })
    return in_maps


def run(inputs, trace=False, trace_kwargs=None):
    """Returns (full complex64 output [64, 32400], BassKernelResults)."""
    from concourse.bass_utils import run_bass_kernel_spmd

    in_maps = _prep_in_maps(**inputs)
    nc = _get_nc()
    res = run_bass_kernel_spmd(
        nc,
        in_maps,
        core_ids=list(range(NCORES)),
        trace=trace,
        **(trace_kwargs or {}),
    )
    dsc = np.float32(1.0 / (SCALE_M * SCALE_B))
    parts = []
    for c in range(NCORES):
        o = res.results[c]["out"]
        parts.append(o[0:64, :] + 1j * o[64:128, :].astype(np.complex64))
    full = (np.concatenate(parts, axis=1)[:, :D] * dsc).astype(np.complex64)
    return full, res


def kernel(**inputs) -> np.ndarray:
    out, _ = run(inputs, trace=False)
    return out


# revision 9
# speedup vs baseline: 1.3333x; 1.0736x over previous
"""ComplexDenseSO2 Trainium2 kernel (v3).

Computes out = (X @ conj(B)^T * w) @ B for complex X [64, 32400],
B [2048, 32400], w [2048], given as separate re/im fp32 planes.

Strategy (tensor-parallel over D across 8 cores):
  - Fold w into the first-matmul operand on the host:
    M = diag(w) @ conj(B), so mm1 output IS Y = X @ M^T.
  - Pad D 32400 -> 32768; core c owns d-slice [c*4096, (c+1)*4096).
  - mm1 merges re/im into ONE PSUM accumulator using two stationary
    variants S_A = (Xr|Xi), S_B = (-Xi|Xr): rows 0:64 = Yr, 64:128 = Yi.
    k is processed in 4 QUARTERS of 512 (1 PSUM bank each) so the four
    128KB AllReduces trigger early and pipeline behind each other while
    mm1/mm2 compute continues.
  - Per quarter: PSUM -> SBUF f32, PE-transpose to [k, j], cast fp16,
    one 128KB DMA to DRAM, AllReduce(add) over the 8 cores.
  - mm2: k-outer with all 8 PSUM banks holding the full [128, 4096]
    f32 output; stationaries ytA = Y^T, ytB = (-Yi^T|Yr^T) built from
    the AR outputs; rhs B tiles streamed as 1MB fully-contiguous DMAs.
  - DMA engine split: input streams (X, M, B) issue on nc.sync;
    AR-coupled transfers (arin writes, arout reads) issue on
    nc.scalar with explicit ordering deps so the scheduler cannot
    park an AR-output read in front of a later AR-input write.
  - Host pre-lays out every tensor so each big DMA is >=1MB with
    >=8KB contiguous bytes per partition line.
  - fp16 operands use power-of-2 prescales (M*1024, B*256) to stay
    clear of fp16 subnormals; the epilogue descales by 2^-18.
"""

import sys

if "/opt/trn_rl_repo" not in sys.path:
    sys.path.insert(0, "/opt/trn_rl_repo")

import numpy as np

B_, K, D = 64, 2048, 32400
NCORES = 8
DP = 32768
DL = DP // NCORES  # 4096
NDT = DL // 128    # 32 d-tiles
NQ = 2             # k-halves
KW = K // NQ       # 1024 k per half
NKB = K // 128     # 16 k-blocks
MCH = 8            # d-tiles per M DMA chunk (2MB per component)

SCALE_M = 1024.0
SCALE_B = 256.0

_nc_cache = {}


def build_nc(n_cores=NCORES):
    import concourse.mybir as mybir
    from concourse import bacc
    import concourse.tile as tile
    from concourse.masks import make_identity
    from concourse.tile_rust import add_dep_helper

    fp = mybir.dt.float16
    f32 = mybir.dt.float32

    nc = bacc.Bacc(
        trn_type="TRN2",
        target_bir_lowering=False,
        debug=False,
        num_devices=n_cores,
    )
    # xa[p, dt*128+j] = S_A[dt*128+p, j]; S_A = (Xr|Xi) along j
    xa = nc.dram_tensor("xa", [128, DL], fp, kind="ExternalInput")
    xb = nc.dram_tensor("xb", [128, DL], fp, kind="ExternalInput")
    # mrh[p, (q*NDT+dt)*KW + kk] = Mr[q*KW+kk, lo+dt*128+p]
    mrh = nc.dram_tensor("mrh", [128, NQ * NDT * KW], fp, kind="ExternalInput")
    mih = nc.dram_tensor("mih", [128, NQ * NDT * KW], fp, kind="ExternalInput")
    # bnr[p, kb*DL + dd] = Br[kb*128+p, lo+dd] * SCALE_B
    bnr = nc.dram_tensor("bnr", [128, NKB * DL], fp, kind="ExternalInput")
    bni = nc.dram_tensor("bni", [128, NKB * DL], fp, kind="ExternalInput")
    out = nc.dram_tensor("out", [128, DL], f32, kind="ExternalOutput")

    with tile.TileContext(nc) as tc:
        with (
            tc.tile_pool(name="sb", bufs=1) as sb,
            tc.tile_pool(name="sbx", bufs=1) as sbx,
            tc.tile_pool(name="ps", bufs=1, space="PSUM") as ps,
            tc.tile_pool(name="dram", bufs=1, space="DRAM") as dram,
        ):
            ident = sbx.tile([128, 128], f32, tag="ident")
            make_identity(nc, ident)

            xa_s = sbx.tile([128, DL], fp, tag="xa_s", name="xa_s")
            nc.sync.dma_start(out=xa_s, in_=xa.ap())
            xb_s = sbx.tile([128, DL], fp, tag="xb_s", name="xb_s")
            nc.sync.dma_start(out=xb_s, in_=xb.ap())

            # AR payload layout is the SBUF-native [128, KW] (the AllReduce
            # is elementwise, so any consistent layout works) -- this keeps
            # the arin write and arout read fully contiguous per partition.
            arin = []
            arout = []
            for q in range(NQ):
                ai = dram.tile([128, KW], fp, tag=f"arin{q}", name=f"arin{q}")
                ao = dram.tile(
                    [128, KW], fp, tag=f"arout{q}", name=f"arout{q}",
                    addr_space="Shared",
                )
                arin.append(ai)
                arout.append(ao)

            # ---------------- mm1: four k-quarters ----------------
            scalar_dmas = []  # for explicit ordering on the scalar queue
            for q in range(NQ):
                acc0 = ps.tile([128, 512], f32, tag=f"p{2 * q}", name="acc0", bufs=1)
                acc1 = ps.tile([128, 512], f32, tag=f"p{2 * q + 1}", name="acc1", bufs=1)
                for g in range(NDT // MCH):
                    base = (q * NDT + g * MCH) * KW
                    mr_t = sb.tile([128, MCH * KW], fp, tag="mr", name="mr", bufs=2)
                    nc.sync.dma_start(out=mr_t, in_=mrh[:, base : base + MCH * KW])
                    mi_t = sb.tile([128, MCH * KW], fp, tag="mi", name="mi", bufs=2)
                    nc.sync.dma_start(out=mi_t, in_=mih[:, base : base + MCH * KW])
                    for t in range(MCH):
                        dt = g * MCH + t
                        st, sp = dt == 0, dt == NDT - 1
                        xs_a = xa_s[:, dt * 128 : (dt + 1) * 128]
                        xs_b = xb_s[:, dt * 128 : (dt + 1) * 128]
                        ts = t * KW
                        nc.tensor.matmul(acc0, lhsT=xs_a, rhs=mr_t[:, ts : ts + 512], start=st, stop=False)
                        nc.tensor.matmul(acc1, lhsT=xs_a, rhs=mr_t[:, ts + 512 : ts + 1024], start=st, stop=False)
                        nc.tensor.matmul(acc0, lhsT=xs_b, rhs=mi_t[:, ts : ts + 512], start=False, stop=sp)
                        nc.tensor.matmul(acc1, lhsT=xs_b, rhs=mi_t[:, ts + 512 : ts + 1024], start=False, stop=sp)

                # Evacuate Y[j, half], transpose to [k, j], ship to AR.
                pscr = sb.tile([128, KW], f32, tag="pscr", name="pscr", bufs=2)
                nc.vector.tensor_copy(pscr[:, 0:512], acc0)
                nc.scalar.copy(pscr[:, 512:1024], acc1)
                yt_sb = sb.tile([128, KW], fp, tag="yt_sb", name="yt_sb", bufs=2)
                for b in range(KW // 128):
                    tp = ps.tile([128, 128], f32, tag=f"p{4 + (b % 2)}", name="tp", bufs=1)
                    nc.tensor.transpose(tp, pscr[:, b * 128 : (b + 1) * 128], ident)
                    nc.vector.tensor_copy(yt_sb[:, b * 128 : (b + 1) * 128], tp)
                w_inst = nc.scalar.dma_start(out=arin[q], in_=yt_sb)
                scalar_dmas.append(w_inst)
                nc.gpsimd.collective_compute(
                    "AllReduce",
                    mybir.AluOpType.add,
                    ins=[arin[q].opt()],
                    outs=[arout[q].opt()],
                    replica_groups=[list(range(n_cores))],
                )

            # ---------------- mm2 stationaries from AR outputs ----------------
            ytA = []
            ytB = []
            for q in range(NQ):
                a_t = sbx.tile([128, KW], fp, tag=f"ytA{q}", name=f"ytA{q}")
                r_inst = nc.scalar.dma_start(out=a_t, in_=arout[q])
                scalar_dmas.append(r_inst)
                b_t = sbx.tile([128, KW], fp, tag=f"ytB{q}", name=f"ytB{q}")
                for b in range(KW // 128):
                    o = b * 128
                    nc.vector.tensor_scalar_mul(b_t[:, o : o + 64], a_t[:, o + 64 : o + 128], -1.0)
                    nc.vector.tensor_copy(b_t[:, o + 64 : o + 128], a_t[:, o : o + 64])
                ytA.append(a_t)
                ytB.append(b_t)

            # Pin the scalar-queue order: every arout read sits after every
            # arin write, and the queue follows emission order. Without this
            # the scheduler can park an AR-output read (waiting on AR k) in
            # front of a later AR-input write, stalling the next AR.
            from_i = None
            for inst in scalar_dmas:
                if from_i is not None:
                    add_dep_helper(inst.ins, from_i.ins, sync=False,
                                   reason="scalar DMA queue order")
                from_i = inst

            # ---------------- mm2: k-outer, 8 PSUM banks ----------------
            pos = []
            for i in range(8):
                po = ps.tile([128, 512], f32, tag=f"p{i}", name=f"po{i}", bufs=1)
                pos.append(po)
            for kp in range(NKB // 2):
                br_t = sb.tile([128, 2 * DL], fp, tag="br", name="br", bufs=2)
                nc.sync.dma_start(out=br_t, in_=bnr[:, 2 * kp * DL : (2 * kp + 2) * DL])
                bi_t = sb.tile([128, 2 * DL], fp, tag="bi", name="bi", bufs=2)
                nc.sync.dma_start(out=bi_t, in_=bni[:, 2 * kp * DL : (2 * kp + 2) * DL])
                for j in range(2):
                    kb = 2 * kp + j
                    q, b = kb // (KW // 128), kb % (KW // 128)
                    ya = ytA[q][:, b * 128 : (b + 1) * 128]
                    yb = ytB[q][:, b * 128 : (b + 1) * 128]
                    st, sp = kb == 0, kb == NKB - 1
                    jo = j * DL
                    for i in range(8):
                        nc.tensor.matmul(pos[i], lhsT=ya, rhs=br_t[:, jo + i * 512 : jo + (i + 1) * 512], start=st, stop=False)
                    for i in range(8):
                        nc.tensor.matmul(pos[i], lhsT=yb, rhs=bi_t[:, jo + i * 512 : jo + (i + 1) * 512], start=False, stop=sp)

            # Descale by 1/(SCALE_M*SCALE_B) happens on the host during
            # output assembly; here only evacuate PSUM -> SBUF.
            osb = sb.tile([128, DL], f32, tag="osb", name="osb", bufs=1)
            for i in range(8):
                if i % 2 == 0:
                    nc.vector.tensor_copy(osb[:, i * 512 : (i + 1) * 512], pos[i])
                else:
                    nc.scalar.copy(osb[:, i * 512 : (i + 1) * 512], pos[i])
            nc.sync.dma_start(out=out[:, 0 : DL // 2], in_=osb[:, 0 : DL // 2])
            nc.sync.dma_start(out=out[:, DL // 2 : DL], in_=osb[:, DL // 2 : DL])

    nc.compile()
    return nc


def _get_nc(n_cores=NCORES):
    if n_cores not in _nc_cache:
        _nc_cache[n_cores] = build_nc(n_cores)
    return _nc_cache[n_cores]


def _prep_in_maps(X_re, X_im, bases_re, bases_im, weight_re, weight_im):
    cdt = np.float16
    f32 = np.float32
    X_re = np.asarray(X_re, f32)
    X_im = np.asarray(X_im, f32)
    bases_re = np.asarray(bases_re, f32)
    bases_im = np.asarray(bases_im, f32)
    wr = np.asarray(weight_re, f32)[:, None]
    wi = np.asarray(weight_im, f32)[:, None]

    # M = diag(w) @ conj(B): Mr = wr*Br + wi*Bi ; Mi = wi*Br - wr*Bi
    mr = (wr * bases_re + wi * bases_im) * f32(SCALE_M)
    mi = (wi * bases_re - wr * bases_im) * f32(SCALE_M)
    bsr = bases_re * f32(SCALE_B)
    bsi = bases_im * f32(SCALE_B)

    in_maps = []
    for c in range(NCORES):
        lo = c * DL
        hi = min((c + 1) * DL, D)
        n = hi - lo

        xat = np.zeros((DL, 128), f32)
        xbt = np.zeros((DL, 128), f32)
        if n > 0:
            xat[:n, 0:64] = X_re[:, lo:hi].T
            xat[:n, 64:128] = X_im[:, lo:hi].T
            xbt[:n, 0:64] = -X_im[:, lo:hi].T
            xbt[:n, 64:128] = X_re[:, lo:hi].T
        # [DL,128] -> [128, DL] with xa[p, dt*128+j] = xat[dt*128+p, j]
        xa = xat.reshape(NDT, 128, 128).transpose(1, 0, 2).reshape(128, DL).astype(cdt)
        xb = xbt.reshape(NDT, 128, 128).transpose(1, 0, 2).reshape(128, DL).astype(cdt)

        def m_layout(m):
            mp = np.zeros((K, DL), f32)
            if n > 0:
                mp[:, :n] = m[:, lo:hi]
            # mrh[p, (q*NDT+dt)*KW + kk] = mp[q*KW+kk, dt*128+p]
            t = mp.reshape(NQ, KW, NDT, 128)
            return t.transpose(3, 0, 2, 1).reshape(128, NQ * NDT * KW).astype(cdt)

        def b_layout(bm):
            bp = np.zeros((K, DL), f32)
            if n > 0:
                bp[:, :n] = bm[:, lo:hi]
            # bnr[p, kb*DL + dd] = bp[kb*128+p, dd]
            t = bp.reshape(NKB, 128, DL)
            return t.transpose(1, 0, 2).reshape(128, NKB * DL).astype(cdt)

        in_maps.append({
            "xa": xa,
            "xb": xb,
            "mrh": m_layout(mr),
            "mih": m_layout(mi),
            "bnr": b_layout(bsr),
            "bni": b_layout(bsi),
        })
    return in_maps


def run(inputs, trace=False, trace_kwargs=None):
    """Returns (full complex64 output [64, 32400], BassKernelResults)."""
    from concourse.bass_utils import run_bass_kernel_spmd

    in_maps = _prep_in_maps(**inputs)
    nc = _get_nc()
    res = run_bass_kernel_spmd(
        nc,
        in_maps,
        core_ids=list(range(NCORES)),
        trace=trace,
        **(trace_kwargs or {}),
    )
    dsc = np.float32(1.0 / (SCALE_M * SCALE_B))
    parts = []
    for c in range(NCORES):
        o = res.results[c]["out"]
        parts.append(o[0:64, :] + 1j * o[64:128, :].astype(np.complex64))
    full = (np.concatenate(parts, axis=1)[:, :D] * dsc).astype(np.complex64)
    return full, res


def kernel(**inputs) -> np.ndarray:
    out, _ = run(inputs, trace=False)
    return out
